# revision 26
# baseline (speedup 1.0000x reference)
"""Trainium2 Bass kernel: batched 2D DCT-II (unnormalized), x: (8, 2048, 2048) f32.

Math: per image X, the unnormalized 2D DCT-II is Z = C @ X @ C^T with
C[k,n] = cos(pi*(2n+1)*k/(2N)).  Let D = C^T.  Using the PE's
out = lhsT.T @ rhs semantics with the *data* as the stationary operand and D as
the moving operand, the two passes chain with no transposes:
    pass 1:  T = X^T @ D      (lhsT = X tiles,  rhs = D)   -> T[c, f]
    pass 2:  Z = T^T @ D      (lhsT = T tiles,  rhs = D)   -> Z = D^T X D = C X C^T

Sharding: batch dim 8 -> one image per NeuronCore (data parallel, no comms).

Dtype modes:
  "f32r"  - single fp32r matmul per term (full PE rate; TF32-like 11-bit
            mantissa operand rounding; ~2e-4 relative-to-absmax error).
  "split" - hi/lo bf16 decomposition, 3 matmuls per term (~5e-6 error, 3x cost).
"""

import numpy as np
from contextlib import ExitStack

import concourse.bass as bass
import concourse.bacc as bacc
import concourse.tile as tile
from concourse import mybir
from concourse.bass_utils import run_bass_kernel_spmd

F32 = mybir.dt.float32
F32R = mybir.dt.float32r
BF16 = mybir.dt.bfloat16

import os

MODE = os.environ.get("DCT_MODE", "bfly")  # "bfly", "f32r", or "split"

B = 8          # batch == n_cores
N = 2048       # image is N x N
P = 128        # partitions
KT = N // P    # 16 k-tiles along any contraction
FC = 512       # chunk width (pass-1 f-chunk, pass-2 g-chunk, PSUM bank)
NFC = N // FC  # 4 chunks
H = N // 2     # butterfly half size
KT2 = H // P   # 8 k-tiles at half contraction


def _round_f32r(a: np.ndarray) -> np.ndarray:
    """fp32r = round-to-nearest, 11 explicit mantissa bits (drop low 12)."""
    b = np.ascontiguousarray(a, dtype=np.float32).view(np.uint32)
    r = ((b + np.uint32(0x800)) & np.uint32(0xFFFFF000)).view(np.float32)
    return r


def _split_bf16(a: np.ndarray):
    import ml_dtypes

    hi = a.astype(ml_dtypes.bfloat16)
    lo = (a - hi.astype(np.float32)).astype(ml_dtypes.bfloat16)
    return hi, lo


def _dct_matrix_d() -> np.ndarray:
    # D[n, k] = cos(pi * (2n+1) * k / (2N)), exact in float64
    n = np.arange(N, dtype=np.float64)[:, None]
    k = np.arange(N, dtype=np.float64)[None, :]
    d = np.cos(np.pi * (2.0 * n + 1.0) * k / (2.0 * N))
    return d.astype(np.float32)


def _build_f32r() -> bass.Bass:
    """fp32r two-pass DCT with the intermediate T round-tripped via DRAM.

    Pass 1 streams X once (one column-block per chain, all 4 f-chunks while
    the block is resident).  T chunks are written back to a DRAM scratch and
    re-streamed as pass-2 stationary tiles.  D stays resident in SBUF.
    """
    nc = bacc.Bacc(None, target_bir_lowering=False)
    x_ext = nc.declare_dram_parameter("x", [N, N], F32R, isOutput=False)
    d_ext = nc.declare_dram_parameter("d", [N, N], F32R, isOutput=False)
    z_ext = nc.declare_dram_parameter("z", [N, N], F32, isOutput=True)

    with ExitStack() as ctx:
        tc = ctx.enter_context(tile.TileContext(nc))
        d_pool = ctx.enter_context(tc.tile_pool(name="d", bufs=1))
        x_pool = ctx.enter_context(tc.tile_pool(name="x", bufs=3))
        t_pool = ctx.enter_context(tc.tile_pool(name="t", bufs=6))
        z_pool = ctx.enter_context(tc.tile_pool(name="z", bufs=3))
        dram = ctx.enter_context(tc.tile_pool(name="dram", bufs=1, space="DRAM"))
        ps1 = ctx.enter_context(tc.tile_pool(name="ps1", bufs=4, space="PSUM"))
        ps2 = ctx.enter_context(tc.tile_pool(name="ps2", bufs=4, space="PSUM"))

        t_dram = dram.tile([N, N], F32R, name="t_dram")

        # First column-block of X loads before D so pass 1 starts early.
        d_sb = [
            d_pool.tile([P, N], F32R, tag=f"d{t}", name=f"d{t}") for t in range(KT)
        ]

        def load_x(cb):
            xt = x_pool.tile([P, N], F32R, tag="x", name="xt")
            nc.sync.dma_start(
                xt[:].rearrange("p (t m) -> p t m", t=KT),
                x_ext[:, cb * P : (cb + 1) * P].rearrange("(t p) m -> p t m", p=P),
            )
            return xt

        x0 = load_x(0)
        # D f-chunk 0 for all 16 row-tiles (pass-1 chain 0 needs only these)
        for fcol in range(NFC):
            for t in range(KT):
                nc.sync.dma_start(
                    d_sb[t][:, fcol * FC : (fcol + 1) * FC],
                    d_ext[t * P : (t + 1) * P, fcol * FC : (fcol + 1) * FC],
                )
            if fcol == 0:
                # remaining D chunks stream behind pass-1 compute
                pass

        # ---- pass 1: per column-block cb, all f-chunks: T[cb,:] = (X^T D)[cb,:]
        for cb in range(KT):
            xt = x0 if cb == 0 else load_x(cb)
            for fc in range(NFC):
                pt = ps1.tile([P, FC], F32, tag="ps1", name="pt")
                for rt in range(KT):
                    nc.tensor.matmul(
                        pt[:],
                        lhsT=xt[:, rt * P : (rt + 1) * P],
                        rhs=d_sb[rt][:, fc * FC : (fc + 1) * FC],
                        start=(rt == 0),
                        stop=(rt == KT - 1),
                    )
                tt = t_pool.tile([P, FC], F32R, tag="t", name="tt")
                nc.vector.tensor_copy(tt[:], pt[:])
                nc.scalar.dma_start(
                    t_dram[cb * P : (cb + 1) * P, fc * FC : (fc + 1) * FC], tt[:]
                )

        # ---- pass 2: per f-block fb: Z[fb,:] = (T^T D)[fb,:]
        for fb in range(KT):
            tf = x_pool.tile([P, N], F32R, tag="x", name="tf")
            nc.sync.dma_start(
                tf[:].rearrange("p (t m) -> p t m", t=KT),
                t_dram[:, fb * P : (fb + 1) * P].rearrange("(t p) m -> p t m", p=P),
            )
            for g in range(NFC):
                pz = ps2.tile([P, FC], F32, tag="ps2", name="pz")
                for ct in range(KT):
                    nc.tensor.matmul(
                        pz[:],
                        lhsT=tf[:, ct * P : (ct + 1) * P],
                        rhs=d_sb[ct][:, g * FC : (g + 1) * FC],
                        start=(ct == 0),
                        stop=(ct == KT - 1),
                    )
                zt = z_pool.tile([P, FC], F32, tag="z", name="zt")
                nc.vector.tensor_copy(zt[:], pz[:])
                nc.scalar.dma_start(
                    z_ext[fb * P : (fb + 1) * P, g * FC : (g + 1) * FC], zt[:]
                )

    nc.finalize()
    return nc


def _build_bfly() -> bass.Bass:
    """Radix-2 even/odd DCT factorization in fp32r: each 1D DCT-II of size N
    becomes two size-N/2 cosine transforms of the folded sequences
    u = x_top + reverse(x_bot), v = x_top - reverse(x_bot):
        y[2j]   = sum_{n<H} u[n] De[n, j],   De[n,j] = cos(pi (2n+1) j / N)
        y[2j+1] = sum_{n<H} v[n] Do[n, j],   Do[n,j] = cos(pi (2n+1)(2j+1) / 2N)
    Halves the matmul work per pass.  Pass-1 folding is done on the host
    (u/v uploaded); pass-2 folding of the intermediate T is done by DVE with a
    reversed-row DMA load.  Outputs are de-interleaved on chip (strided DVE
    writes) + stride-2-row DMA stores, so all DRAM traffic stays contiguous
    per partition.
    """
    nc = bacc.Bacc(None, target_bir_lowering=False)
    u_ext = nc.declare_dram_parameter("u", [H, N], F32R, isOutput=False)
    v_ext = nc.declare_dram_parameter("v", [H, N], F32R, isOutput=False)
    de_ext = nc.declare_dram_parameter("de", [H, H], F32R, isOutput=False)
    do_ext = nc.declare_dram_parameter("do", [H, H], F32R, isOutput=False)
    r_ext = nc.declare_dram_parameter("r", [P, P], F32R, isOutput=False)
    z_ext = nc.declare_dram_parameter("z", [N, N], F32, isOutput=True)

    with ExitStack() as ctx:
        tc = ctx.enter_context(tile.TileContext(nc))
        d_pool = ctx.enter_context(tc.tile_pool(name="d", bufs=1))
        x_pool = ctx.enter_context(tc.tile_pool(name="x", bufs=3))
        t_pool = ctx.enter_context(tc.tile_pool(name="t", bufs=4))
        b_pool = ctx.enter_context(tc.tile_pool(name="b", bufs=3))
        z_pool = ctx.enter_context(tc.tile_pool(name="z", bufs=3))
        dram = ctx.enter_context(tc.tile_pool(name="dram", bufs=1, space="DRAM"))
        # ps1 slots are [P, H] f32 (2 banks) shared between pass-1 accumulators
        # and pass-2 reversal matmuls: 2x2 + ps2 2x2 = 8 banks total
        ps1 = ctx.enter_context(tc.tile_pool(name="ps1", bufs=2, space="PSUM"))
        ps2 = ctx.enter_context(tc.tile_pool(name="ps2", bufs=2, space="PSUM"))

        # T in blocked layout: cols [0,H) = even outputs, [H,2H) = odd
        t_dram = dram.tile([N, N], F32R, name="t_dram")

        de_sb = [
            d_pool.tile([P, H], F32R, tag=f"de{t}", name=f"de{t}")
            for t in range(KT2)
        ]
        do_sb = [
            d_pool.tile([P, H], F32R, tag=f"do{t}", name=f"do{t}")
            for t in range(KT2)
        ]

        def load_block(ext, cb, tag):
            w = x_pool.tile([P, H], F32R, tag=tag, name="w_" + tag)
            nc.sync.dma_start(
                w[:].rearrange("p (t m) -> p t m", t=KT2),
                ext[:, cb * P : (cb + 1) * P].rearrange("(t p) m -> p t m", p=P),
            )
            return w

        # loads in exact first-consumption order: u0, de jc0, de jc1, v0,
        # do jc0, do jc1; the pass-2 reversal matrix r last.
        u0 = load_block(u_ext, 0, "u")
        for jc in range(2):
            for t in range(KT2):
                nc.sync.dma_start(
                    de_sb[t][:, jc * FC : (jc + 1) * FC],
                    de_ext[t * P : (t + 1) * P, jc * FC : (jc + 1) * FC],
                )
        v0 = load_block(v_ext, 0, "v")
        for jc in range(2):
            for t in range(KT2):
                nc.sync.dma_start(
                    do_sb[t][:, jc * FC : (jc + 1) * FC],
                    do_ext[t * P : (t + 1) * P, jc * FC : (jc + 1) * FC],
                )
        r_sb = d_pool.tile([P, P], F32R, tag="r", name="r_sb")
        nc.sync.dma_start(r_sb[:], r_ext[:])

        # ---- pass 1: T_blk[cb, :] ----
        for cb in range(KT):
            ut = u0 if cb == 0 else load_block(u_ext, cb, "u")
            vt = v0 if cb == 0 else load_block(v_ext, cb, "v")
            for half, (wt, dsb) in enumerate(((ut, de_sb), (vt, do_sb))):
                for jc in range(2):
                    pt = ps1.tile([P, FC], F32, tag="acc", name="pt")
                    for rt in range(KT2):
                        nc.tensor.matmul(
                            pt[:],
                            lhsT=wt[:, rt * P : (rt + 1) * P],
                            rhs=dsb[rt][:, jc * FC : (jc + 1) * FC],
                            start=(rt == 0),
                            stop=(rt == KT2 - 1),
                        )
                    tt = t_pool.tile([P, FC], F32R, tag="t", name="tt")
                    nc.vector.tensor_copy(tt[:], pt[:])
                    col0 = half * H + jc * FC
                    nc.scalar.dma_start(
                        t_dram[cb * P : (cb + 1) * P, col0 : col0 + FC], tt[:]
                    )

        # ---- pass 2: fold T over rows, transform, de-interleave out ----
        # bot_rev[c', f] = T[2047-c', f]: partition reversal via one PE matmul
        # with the reversal permutation R (out[m,n] = bot[127-m, n]); the
        # tile-order flip (ct -> 7-ct) via a reversed free-dim view in the add.
        # Software-pipelined: loads run 3 blocks ahead, reversal matmul + DVE
        # fold 2 ahead, so block fb's chains never wait on its fold.
        folded: dict = {}

        def p2_load(fb):
            top = b_pool.tile([P, H], F32R, tag="top", name="top")
            nc.sync.dma_start(
                top[:].rearrange("p (t m) -> p t m", t=KT2),
                t_dram[0:H, fb * P : (fb + 1) * P].rearrange(
                    "(t p) m -> p t m", p=P
                ),
            )
            bot = b_pool.tile([P, H], F32R, tag="bot", name="bot")
            nc.sync.dma_start(
                bot[:].rearrange("p (t m) -> p t m", t=KT2),
                t_dram[H:N, fb * P : (fb + 1) * P].rearrange(
                    "(t p) m -> p t m", p=P
                ),
            )
            folded[fb] = (top, bot)

        def p2_fold(fb):
            top, bot = folded[fb]
            pr = ps1.tile([P, H], F32, tag="acc", name="pr")
            for hc in range(2):
                nc.tensor.matmul(
                    pr[:, hc * FC : (hc + 1) * FC],
                    lhsT=r_sb[:],
                    rhs=bot[:, hc * FC : (hc + 1) * FC],
                    start=True,
                    stop=True,
                )
            # view with tile index reversed: pr_rev[p, ct, m] = pr[p, 7-ct, m]
            pr_rev = pr[:].rearrange("p (t m) -> p t m", t=KT2)[:, ::-1, :]
            top3 = top[:].rearrange("p (t m) -> p t m", t=KT2)
            u2 = b_pool.tile([P, H], F32R, tag="u2", name="u2")
            nc.vector.tensor_add(
                u2[:].rearrange("p (t m) -> p t m", t=KT2), top3, pr_rev
            )
            v2 = b_pool.tile([P, H], F32R, tag="v2", name="v2")
            nc.vector.tensor_sub(
                v2[:].rearrange("p (t m) -> p t m", t=KT2), top3, pr_rev
            )
            folded[fb] = (u2, v2)

        p2_load(0)
        p2_load(1)
        p2_fold(0)
        p2_load(2)
        p2_fold(1)
        for fb in range(KT):
            u2, v2 = folded.pop(fb)
            # f_blk block fb -> actual Z rows (de-interleave rows via stride 2)
            if fb < KT2:
                row0 = 2 * fb * P
                row_stop = row0 + 2 * P
            else:
                row0 = 2 * (fb - KT2) * P + 1
                row_stop = row0 + 2 * P - 1
            for jc in range(2):
                pe_ = ps2.tile([P, FC], F32, tag="pse", name="pe_")
                for ct in range(KT2):
                    nc.tensor.matmul(
                        pe_[:],
                        lhsT=u2[:, ct * P : (ct + 1) * P],
                        rhs=de_sb[ct][:, jc * FC : (jc + 1) * FC],
                        start=(ct == 0),
                        stop=(ct == KT2 - 1),
                    )
                po_ = ps2.tile([P, FC], F32, tag="pso", name="po_")
                for ct in range(KT2):
                    nc.tensor.matmul(
                        po_[:],
                        lhsT=v2[:, ct * P : (ct + 1) * P],
                        rhs=do_sb[ct][:, jc * FC : (jc + 1) * FC],
                        start=(ct == 0),
                        stop=(ct == KT2 - 1),
                    )
                zt = z_pool.tile([P, 2 * FC], F32, tag="z", name="zt")
                # balance the two de-interleave copies across ACT and DVE
                if jc == 0:
                    nc.scalar.copy(zt[:, 0 : 2 * FC : 2], pe_[:])
                    nc.vector.tensor_copy(zt[:, 1 : 2 * FC : 2], po_[:])
                else:
                    nc.vector.tensor_copy(zt[:, 0 : 2 * FC : 2], pe_[:])
                    nc.scalar.copy(zt[:, 1 : 2 * FC : 2], po_[:])
                nc.scalar.dma_start(
                    z_ext[row0:row_stop:2, jc * 2 * FC : (jc + 1) * 2 * FC],
                    zt[:],
                )
            if fb + 3 < KT:
                p2_load(fb + 3)
            if fb + 2 < KT:
                p2_fold(fb + 2)

    nc.finalize()
    return nc


def _build_split() -> bass.Bass:
    """hi/lo bf16 decomposition: each logical matmul = 3 bf16 matmuls
    (Xh Dh + Xh Dl + Xl Dh), accumulated in the same PSUM chain."""
    nc = bacc.Bacc(None, target_bir_lowering=False)
    xh_ext = nc.declare_dram_parameter("xh", [N, N], BF16, isOutput=False)
    xl_ext = nc.declare_dram_parameter("xl", [N, N], BF16, isOutput=False)
    dh_ext = nc.declare_dram_parameter("dh", [N, N], BF16, isOutput=False)
    dl_ext = nc.declare_dram_parameter("dl", [N, N], BF16, isOutput=False)
    z_ext = nc.declare_dram_parameter("z", [N, N], F32, isOutput=True)

    with ExitStack() as ctx:
        tc = ctx.enter_context(tile.TileContext(nc))
        d_pool = ctx.enter_context(tc.tile_pool(name="d", bufs=1))
        x_pool = ctx.enter_context(tc.tile_pool(name="x", bufs=3))
        w_pool = ctx.enter_context(tc.tile_pool(name="w", bufs=3))
        t_pool = ctx.enter_context(tc.tile_pool(name="t", bufs=KT))
        z_pool = ctx.enter_context(tc.tile_pool(name="z", bufs=3))
        ps1 = ctx.enter_context(tc.tile_pool(name="ps1", bufs=4, space="PSUM"))
        ps2 = ctx.enter_context(tc.tile_pool(name="ps2", bufs=4, space="PSUM"))

        dh_sb = [
            d_pool.tile([P, N], BF16, tag=f"dh{t}", name=f"dh{t}")
            for t in range(KT)
        ]
        dl_sb = [
            d_pool.tile([P, N], BF16, tag=f"dl{t}", name=f"dl{t}")
            for t in range(KT)
        ]
        for fcol in range(NFC):
            for t in range(KT):
                nc.sync.dma_start(
                    dh_sb[t][:, fcol * FC : (fcol + 1) * FC],
                    dh_ext[t * P : (t + 1) * P, fcol * FC : (fcol + 1) * FC],
                )
                nc.sync.dma_start(
                    dl_sb[t][:, fcol * FC : (fcol + 1) * FC],
                    dl_ext[t * P : (t + 1) * P, fcol * FC : (fcol + 1) * FC],
                )

        for fc in range(NFC):
            t_tiles = []
            for cb in range(KT):
                xht = x_pool.tile([P, N], BF16, tag="xh", name="xht")
                xlt = x_pool.tile([P, N], BF16, tag="xl", name="xlt")
                for t_, ext in ((xht, xh_ext), (xlt, xl_ext)):
                    nc.sync.dma_start(
                        t_[:].rearrange("p (t m) -> p t m", t=KT),
                        ext[:, cb * P : (cb + 1) * P].rearrange(
                            "(t p) m -> p t m", p=P
                        ),
                    )
                pt = ps1.tile([P, FC], F32, tag="ps1", name="pt")
                nmm = 3 * KT
                i = 0
                for rt in range(KT):
                    dh = dh_sb[rt][:, fc * FC : (fc + 1) * FC]
                    dl = dl_sb[rt][:, fc * FC : (fc + 1) * FC]
                    xh = xht[:, rt * P : (rt + 1) * P]
                    xl = xlt[:, rt * P : (rt + 1) * P]
                    for l_, r_ in ((xh, dh), (xh, dl), (xl, dh)):
                        nc.tensor.matmul(
                            pt[:], lhsT=l_, rhs=r_,
                            start=(i == 0), stop=(i == nmm - 1),
                        )
                        i += 1
                # split T on device: th = bf16(T), tl = bf16(T - th)
                th = t_pool.tile([P, FC], BF16, tag="th", name="th")
                tl = t_pool.tile([P, FC], BF16, tag="tl", name="tl")
                tmp = w_pool.tile([P, FC], F32, tag="tmp", name="tmp")
                nc.vector.tensor_copy(th[:], pt[:])
                nc.scalar.copy(tmp[:], th[:])
                nc.vector.tensor_sub(tmp[:], pt[:], tmp[:])
                nc.vector.tensor_copy(tl[:], tmp[:])
                t_tiles.append((th, tl))

            for fb in range(FC // P):
                for g in range(NFC):
                    pz = ps2.tile([P, FC], F32, tag="ps2", name="pz")
                    nmm = 3 * KT
                    i = 0
                    for ct in range(KT):
                        th, tl = t_tiles[ct]
                        dh = dh_sb[ct][:, g * FC : (g + 1) * FC]
                        dl = dl_sb[ct][:, g * FC : (g + 1) * FC]
                        thb = th[:, fb * P : (fb + 1) * P]
                        tlb = tl[:, fb * P : (fb + 1) * P]
                        for l_, r_ in ((thb, dh), (thb, dl), (tlb, dh)):
                            nc.tensor.matmul(
                                pz[:], lhsT=l_, rhs=r_,
                                start=(i == 0), stop=(i == nmm - 1),
                            )
                            i += 1
                    zt = z_pool.tile([P, FC], F32, tag="z", name="zt")
                    nc.vector.tensor_copy(zt[:], pz[:])
                    row0 = (fc * (FC // P) + fb) * P
                    nc.sync.dma_start(
                        z_ext[row0 : row0 + P, g * FC : (g + 1) * FC], zt[:]
                    )

    nc.finalize()
    return nc


_PROGRAM_CACHE: dict = {}


_BUILDERS = {"f32r": _build_f32r, "bfly": _build_bfly, "split": _build_split}


def _get_program(mode: str) -> bass.Bass:
    if mode not in _PROGRAM_CACHE:
        _PROGRAM_CACHE[mode] = _BUILDERS[mode]()
    return _PROGRAM_CACHE[mode]


def _make_in_maps(x: np.ndarray, mode: str):
    if mode == "f32r":
        dr = _round_f32r(_dct_matrix_d())
        return [{"x": _round_f32r(x[i]), "d": dr} for i in range(B)]
    if mode == "bfly":
        n2 = np.arange(H, dtype=np.float64)[:, None]
        j2 = np.arange(H, dtype=np.float64)[None, :]
        de = _round_f32r(np.cos(np.pi * (2 * n2 + 1) * j2 / N).astype(np.float32))
        do = _round_f32r(
            np.cos(np.pi * (2 * n2 + 1) * (2 * j2 + 1) / (2 * N)).astype(
                np.float32
            )
        )
        r = np.eye(P, dtype=np.float32)[::-1].copy()
        maps = []
        for i in range(B):
            xi = np.asarray(x[i], dtype=np.float32)
            xr = xi[::-1]
            maps.append(
                {
                    "u": _round_f32r(xi[:H] + xr[:H]),
                    "v": _round_f32r(xi[:H] - xr[:H]),
                    "de": de,
                    "do": do,
                    "r": r,
                }
            )
        return maps
    dh, dl = _split_bf16(_dct_matrix_d())
    maps = []
    for i in range(B):
        xh, xl = _split_bf16(np.ascontiguousarray(x[i], dtype=np.float32))
        maps.append({"xh": xh, "xl": xl, "dh": dh, "dl": dl})
    return maps


def kernel(x: np.ndarray) -> np.ndarray:
    x = np.asarray(x)
    assert x.shape == (B, N, N), x.shape
    nc = _get_program(MODE)
    in_maps = _make_in_maps(x, MODE)
    res = run_bass_kernel_spmd(nc, in_maps, list(range(B)))
    out = np.stack([res.results[i]["z"] for i in range(B)], axis=0)
    return out.astype(np.float32, copy=False)


# revision 27
# speedup vs baseline: 1.1374x; 1.1374x over previous
"""Trainium2 Bass kernel: batched 2D DCT-II (unnormalized), x: (8, 2048, 2048) f32.

Math: per image X, the unnormalized 2D DCT-II is Z = C @ X @ C^T with
C[k,n] = cos(pi*(2n+1)*k/(2N)).  Let D = C^T.  Using the PE's
out = lhsT.T @ rhs semantics with the *data* as the stationary operand and D as
the moving operand, the two passes chain with no transposes:
    pass 1:  T = X^T @ D      (lhsT = X tiles,  rhs = D)   -> T[c, f]
    pass 2:  Z = T^T @ D      (lhsT = T tiles,  rhs = D)   -> Z = D^T X D = C X C^T

Sharding: batch dim 8 -> one image per NeuronCore (data parallel, no comms).

Dtype modes:
  "f32r"  - single fp32r matmul per term (full PE rate; TF32-like 11-bit
            mantissa operand rounding; ~2e-4 relative-to-absmax error).
  "split" - hi/lo bf16 decomposition, 3 matmuls per term (~5e-6 error, 3x cost).
"""

import numpy as np
from contextlib import ExitStack

import concourse.bass as bass
import concourse.bacc as bacc
import concourse.tile as tile
from concourse import mybir
from concourse.bass_utils import run_bass_kernel_spmd

F32 = mybir.dt.float32
F32R = mybir.dt.float32r
BF16 = mybir.dt.bfloat16

import os

MODE = os.environ.get("DCT_MODE", "bfly")  # "bfly", "f32r", or "split"

B = 8          # batch == n_cores
N = 2048       # image is N x N
P = 128        # partitions
KT = N // P    # 16 k-tiles along any contraction
FC = 512       # chunk width (pass-1 f-chunk, pass-2 g-chunk, PSUM bank)
NFC = N // FC  # 4 chunks
H = N // 2     # butterfly half size
KT2 = H // P   # 8 k-tiles at half contraction


def _round_f32r(a: np.ndarray) -> np.ndarray:
    """fp32r = round-to-nearest, 11 explicit mantissa bits (drop low 12)."""
    b = np.ascontiguousarray(a, dtype=np.float32).view(np.uint32)
    r = ((b + np.uint32(0x800)) & np.uint32(0xFFFFF000)).view(np.float32)
    return r


def _split_bf16(a: np.ndarray):
    import ml_dtypes

    hi = a.astype(ml_dtypes.bfloat16)
    lo = (a - hi.astype(np.float32)).astype(ml_dtypes.bfloat16)
    return hi, lo


def _dct_matrix_d() -> np.ndarray:
    # D[n, k] = cos(pi * (2n+1) * k / (2N)), exact in float64
    n = np.arange(N, dtype=np.float64)[:, None]
    k = np.arange(N, dtype=np.float64)[None, :]
    d = np.cos(np.pi * (2.0 * n + 1.0) * k / (2.0 * N))
    return d.astype(np.float32)


def _build_f32r() -> bass.Bass:
    """fp32r two-pass DCT with the intermediate T round-tripped via DRAM.

    Pass 1 streams X once (one column-block per chain, all 4 f-chunks while
    the block is resident).  T chunks are written back to a DRAM scratch and
    re-streamed as pass-2 stationary tiles.  D stays resident in SBUF.
    """
    nc = bacc.Bacc(None, target_bir_lowering=False)
    x_ext = nc.declare_dram_parameter("x", [N, N], F32R, isOutput=False)
    d_ext = nc.declare_dram_parameter("d", [N, N], F32R, isOutput=False)
    z_ext = nc.declare_dram_parameter("z", [N, N], F32, isOutput=True)

    with ExitStack() as ctx:
        tc = ctx.enter_context(tile.TileContext(nc))
        d_pool = ctx.enter_context(tc.tile_pool(name="d", bufs=1))
        x_pool = ctx.enter_context(tc.tile_pool(name="x", bufs=3))
        t_pool = ctx.enter_context(tc.tile_pool(name="t", bufs=6))
        z_pool = ctx.enter_context(tc.tile_pool(name="z", bufs=3))
        dram = ctx.enter_context(tc.tile_pool(name="dram", bufs=1, space="DRAM"))
        ps1 = ctx.enter_context(tc.tile_pool(name="ps1", bufs=4, space="PSUM"))
        ps2 = ctx.enter_context(tc.tile_pool(name="ps2", bufs=4, space="PSUM"))

        t_dram = dram.tile([N, N], F32R, name="t_dram")

        # First column-block of X loads before D so pass 1 starts early.
        d_sb = [
            d_pool.tile([P, N], F32R, tag=f"d{t}", name=f"d{t}") for t in range(KT)
        ]

        def load_x(cb):
            xt = x_pool.tile([P, N], F32R, tag="x", name="xt")
            nc.sync.dma_start(
                xt[:].rearrange("p (t m) -> p t m", t=KT),
                x_ext[:, cb * P : (cb + 1) * P].rearrange("(t p) m -> p t m", p=P),
            )
            return xt

        x0 = load_x(0)
        # D f-chunk 0 for all 16 row-tiles (pass-1 chain 0 needs only these)
        for fcol in range(NFC):
            for t in range(KT):
                nc.sync.dma_start(
                    d_sb[t][:, fcol * FC : (fcol + 1) * FC],
                    d_ext[t * P : (t + 1) * P, fcol * FC : (fcol + 1) * FC],
                )
            if fcol == 0:
                # remaining D chunks stream behind pass-1 compute
                pass

        # ---- pass 1: per column-block cb, all f-chunks: T[cb,:] = (X^T D)[cb,:]
        for cb in range(KT):
            xt = x0 if cb == 0 else load_x(cb)
            for fc in range(NFC):
                pt = ps1.tile([P, FC], F32, tag="ps1", name="pt")
                for rt in range(KT):
                    nc.tensor.matmul(
                        pt[:],
                        lhsT=xt[:, rt * P : (rt + 1) * P],
                        rhs=d_sb[rt][:, fc * FC : (fc + 1) * FC],
                        start=(rt == 0),
                        stop=(rt == KT - 1),
                    )
                tt = t_pool.tile([P, FC], F32R, tag="t", name="tt")
                nc.vector.tensor_copy(tt[:], pt[:])
                nc.scalar.dma_start(
                    t_dram[cb * P : (cb + 1) * P, fc * FC : (fc + 1) * FC], tt[:]
                )

        # ---- pass 2: per f-block fb: Z[fb,:] = (T^T D)[fb,:]
        for fb in range(KT):
            tf = x_pool.tile([P, N], F32R, tag="x", name="tf")
            nc.sync.dma_start(
                tf[:].rearrange("p (t m) -> p t m", t=KT),
                t_dram[:, fb * P : (fb + 1) * P].rearrange("(t p) m -> p t m", p=P),
            )
            for g in range(NFC):
                pz = ps2.tile([P, FC], F32, tag="ps2", name="pz")
                for ct in range(KT):
                    nc.tensor.matmul(
                        pz[:],
                        lhsT=tf[:, ct * P : (ct + 1) * P],
                        rhs=d_sb[ct][:, g * FC : (g + 1) * FC],
                        start=(ct == 0),
                        stop=(ct == KT - 1),
                    )
                zt = z_pool.tile([P, FC], F32, tag="z", name="zt")
                nc.vector.tensor_copy(zt[:], pz[:])
                nc.scalar.dma_start(
                    z_ext[fb * P : (fb + 1) * P, g * FC : (g + 1) * FC], zt[:]
                )

    nc.finalize()
    return nc


def _build_bfly() -> bass.Bass:
    """Radix-2 even/odd DCT factorization in fp32r: each 1D DCT-II of size N
    becomes two size-N/2 cosine transforms of the folded sequences
    u = x_top + reverse(x_bot), v = x_top - reverse(x_bot):
        y[2j]   = sum_{n<H} u[n] De[n, j],   De[n,j] = cos(pi (2n+1) j / N)
        y[2j+1] = sum_{n<H} v[n] Do[n, j],   Do[n,j] = cos(pi (2n+1)(2j+1) / 2N)
    Halves the matmul work per pass.  Pass-1 folding is done on the host
    (u/v uploaded); pass-2 folding of the intermediate T is done by DVE with a
    reversed-row DMA load.  Outputs are de-interleaved on chip (strided DVE
    writes) + stride-2-row DMA stores, so all DRAM traffic stays contiguous
    per partition.
    """
    nc = bacc.Bacc(None, target_bir_lowering=False)
    u_ext = nc.declare_dram_parameter("u", [H, N], F32R, isOutput=False)
    v_ext = nc.declare_dram_parameter("v", [H, N], F32R, isOutput=False)
    de_ext = nc.declare_dram_parameter("de", [H, H], F32R, isOutput=False)
    do_ext = nc.declare_dram_parameter("do", [H, H], F32R, isOutput=False)
    r_ext = nc.declare_dram_parameter("r", [P, P], F32R, isOutput=False)
    z_ext = nc.declare_dram_parameter("z", [N, N], F32, isOutput=True)

    with ExitStack() as ctx:
        tc = ctx.enter_context(tile.TileContext(nc))
        d_pool = ctx.enter_context(tc.tile_pool(name="d", bufs=1))
        x_pool = ctx.enter_context(tc.tile_pool(name="x", bufs=3))
        t_pool = ctx.enter_context(tc.tile_pool(name="t", bufs=4))
        b_pool = ctx.enter_context(tc.tile_pool(name="b", bufs=3))
        z_pool = ctx.enter_context(tc.tile_pool(name="z", bufs=3))
        dram = ctx.enter_context(tc.tile_pool(name="dram", bufs=1, space="DRAM"))
        # ps1 slots are [P, H] f32 (2 banks) shared between pass-1 accumulators
        # and pass-2 reversal matmuls: 2x2 + ps2 2x2 = 8 banks total
        ps1 = ctx.enter_context(tc.tile_pool(name="ps1", bufs=2, space="PSUM"))
        ps2 = ctx.enter_context(tc.tile_pool(name="ps2", bufs=2, space="PSUM"))

        # T in blocked layout: cols [0,H) = even outputs, [H,2H) = odd
        t_dram = dram.tile([N, N], F32R, name="t_dram")

        de_sb = [
            d_pool.tile([P, H], F32R, tag=f"de{t}", name=f"de{t}")
            for t in range(KT2)
        ]
        do_sb = [
            d_pool.tile([P, H], F32R, tag=f"do{t}", name=f"do{t}")
            for t in range(KT2)
        ]

        def load_block(ext, cb, tag):
            w = x_pool.tile([P, H], F32R, tag=tag, name="w_" + tag)
            nc.sync.dma_start(
                w[:].rearrange("p (t m) -> p t m", t=KT2),
                ext[:, cb * P : (cb + 1) * P].rearrange("(t p) m -> p t m", p=P),
            )
            return w

        # loads in exact first-consumption order: u0, de jc0, de jc1, v0,
        # do jc0, do jc1; the pass-2 reversal matrix r last.
        u0 = load_block(u_ext, 0, "u")
        for jc in range(2):
            for t in range(KT2):
                nc.sync.dma_start(
                    de_sb[t][:, jc * FC : (jc + 1) * FC],
                    de_ext[t * P : (t + 1) * P, jc * FC : (jc + 1) * FC],
                )
        v0 = load_block(v_ext, 0, "v")
        for jc in range(2):
            for t in range(KT2):
                nc.sync.dma_start(
                    do_sb[t][:, jc * FC : (jc + 1) * FC],
                    do_ext[t * P : (t + 1) * P, jc * FC : (jc + 1) * FC],
                )
        r_sb = d_pool.tile([P, P], F32R, tag="r", name="r_sb")
        nc.sync.dma_start(r_sb[:], r_ext[:])

        # ---- pass 1: T_blk[cb, :] ----
        for cb in range(KT):
            ut = u0 if cb == 0 else load_block(u_ext, cb, "u")
            vt = v0 if cb == 0 else load_block(v_ext, cb, "v")
            for half, (wt, dsb) in enumerate(((ut, de_sb), (vt, do_sb))):
                for jc in range(2):
                    pt = ps1.tile([P, FC], F32, tag="acc", name="pt")
                    for rt in range(KT2):
                        nc.tensor.matmul(
                            pt[:],
                            lhsT=wt[:, rt * P : (rt + 1) * P],
                            rhs=dsb[rt][:, jc * FC : (jc + 1) * FC],
                            start=(rt == 0),
                            stop=(rt == KT2 - 1),
                        )
                    tt = t_pool.tile([P, FC], F32R, tag="t", name="tt")
                    nc.vector.tensor_copy(tt[:], pt[:])
                    col0 = half * H + jc * FC
                    nc.scalar.dma_start(
                        t_dram[cb * P : (cb + 1) * P, col0 : col0 + FC], tt[:]
                    )

        # ---- pass 2: fold T over rows, transform, de-interleave out ----
        # bot_rev[c', f] = T[2047-c', f]: partition reversal via one PE matmul
        # with the reversal permutation R (out[m,n] = bot[127-m, n]); the
        # tile-order flip (ct -> 7-ct) via a reversed free-dim view in the add.
        # Software-pipelined: loads run 3 blocks ahead, reversal matmul + DVE
        # fold 2 ahead, so block fb's chains never wait on its fold.
        folded: dict = {}

        def p2_load(fb):
            top = b_pool.tile([P, H], F32R, tag="top", name="top")
            nc.sync.dma_start(
                top[:].rearrange("p (t m) -> p t m", t=KT2),
                t_dram[0:H, fb * P : (fb + 1) * P].rearrange(
                    "(t p) m -> p t m", p=P
                ),
            )
            bot = b_pool.tile([P, H], F32R, tag="bot", name="bot")
            nc.sync.dma_start(
                bot[:].rearrange("p (t m) -> p t m", t=KT2),
                t_dram[H:N, fb * P : (fb + 1) * P].rearrange(
                    "(t p) m -> p t m", p=P
                ),
            )
            folded[fb] = (top, bot)

        def p2_fold(fb):
            top, bot = folded[fb]
            pr = ps1.tile([P, H], F32, tag="acc", name="pr")
            for hc in range(2):
                nc.tensor.matmul(
                    pr[:, hc * FC : (hc + 1) * FC],
                    lhsT=r_sb[:],
                    rhs=bot[:, hc * FC : (hc + 1) * FC],
                    start=True,
                    stop=True,
                )
            # view with tile index reversed: pr_rev[p, ct, m] = pr[p, 7-ct, m]
            pr_rev = pr[:].rearrange("p (t m) -> p t m", t=KT2)[:, ::-1, :]
            top3 = top[:].rearrange("p (t m) -> p t m", t=KT2)
            u2 = b_pool.tile([P, H], F32R, tag="u2", name="u2")
            nc.vector.tensor_add(
                u2[:].rearrange("p (t m) -> p t m", t=KT2), top3, pr_rev
            )
            v2 = b_pool.tile([P, H], F32R, tag="v2", name="v2")
            nc.vector.tensor_sub(
                v2[:].rearrange("p (t m) -> p t m", t=KT2), top3, pr_rev
            )
            folded[fb] = (u2, v2)

        p2_load(0)
        p2_load(1)
        p2_fold(0)
        p2_load(2)
        p2_fold(1)
        for fb in range(KT):
            u2, v2 = folded.pop(fb)
            # f_blk block fb -> actual Z rows (de-interleave rows via stride 2)
            if fb < KT2:
                row0 = 2 * fb * P
                row_stop = row0 + 2 * P
            else:
                row0 = 2 * (fb - KT2) * P + 1
                row_stop = row0 + 2 * P - 1
            for jc in range(2):
                pe_ = ps2.tile([P, FC], F32, tag="pse", name="pe_")
                for ct in range(KT2):
                    nc.tensor.matmul(
                        pe_[:],
                        lhsT=u2[:, ct * P : (ct + 1) * P],
                        rhs=de_sb[ct][:, jc * FC : (jc + 1) * FC],
                        start=(ct == 0),
                        stop=(ct == KT2 - 1),
                    )
                po_ = ps2.tile([P, FC], F32, tag="pso", name="po_")
                for ct in range(KT2):
                    nc.tensor.matmul(
                        po_[:],
                        lhsT=v2[:, ct * P : (ct + 1) * P],
                        rhs=do_sb[ct][:, jc * FC : (jc + 1) * FC],
                        start=(ct == 0),
                        stop=(ct == KT2 - 1),
                    )
                zt = z_pool.tile([P, 2 * FC], F32, tag="z", name="zt")
                nc.scalar.copy(zt[:, 0 : 2 * FC : 2], pe_[:])
                nc.vector.tensor_copy(zt[:, 1 : 2 * FC : 2], po_[:])
                nc.scalar.dma_start(
                    z_ext[row0:row_stop:2, jc * 2 * FC : (jc + 1) * 2 * FC],
                    zt[:],
                )
            if fb + 3 < KT:
                p2_load(fb + 3)
            if fb + 2 < KT:
                p2_fold(fb + 2)

    nc.finalize()
    return nc


def _build_split() -> bass.Bass:
    """hi/lo bf16 decomposition: each logical matmul = 3 bf16 matmuls
    (Xh Dh + Xh Dl + Xl Dh), accumulated in the same PSUM chain."""
    nc = bacc.Bacc(None, target_bir_lowering=False)
    xh_ext = nc.declare_dram_parameter("xh", [N, N], BF16, isOutput=False)
    xl_ext = nc.declare_dram_parameter("xl", [N, N], BF16, isOutput=False)
    dh_ext = nc.declare_dram_parameter("dh", [N, N], BF16, isOutput=False)
    dl_ext = nc.declare_dram_parameter("dl", [N, N], BF16, isOutput=False)
    z_ext = nc.declare_dram_parameter("z", [N, N], F32, isOutput=True)

    with ExitStack() as ctx:
        tc = ctx.enter_context(tile.TileContext(nc))
        d_pool = ctx.enter_context(tc.tile_pool(name="d", bufs=1))
        x_pool = ctx.enter_context(tc.tile_pool(name="x", bufs=3))
        w_pool = ctx.enter_context(tc.tile_pool(name="w", bufs=3))
        t_pool = ctx.enter_context(tc.tile_pool(name="t", bufs=KT))
        z_pool = ctx.enter_context(tc.tile_pool(name="z", bufs=3))
        ps1 = ctx.enter_context(tc.tile_pool(name="ps1", bufs=4, space="PSUM"))
        ps2 = ctx.enter_context(tc.tile_pool(name="ps2", bufs=4, space="PSUM"))

        dh_sb = [
            d_pool.tile([P, N], BF16, tag=f"dh{t}", name=f"dh{t}")
            for t in range(KT)
        ]
        dl_sb = [
            d_pool.tile([P, N], BF16, tag=f"dl{t}", name=f"dl{t}")
            for t in range(KT)
        ]
        for fcol in range(NFC):
            for t in range(KT):
                nc.sync.dma_start(
                    dh_sb[t][:, fcol * FC : (fcol + 1) * FC],
                    dh_ext[t * P : (t + 1) * P, fcol * FC : (fcol + 1) * FC],
                )
                nc.sync.dma_start(
                    dl_sb[t][:, fcol * FC : (fcol + 1) * FC],
                    dl_ext[t * P : (t + 1) * P, fcol * FC : (fcol + 1) * FC],
                )

        for fc in range(NFC):
            t_tiles = []
            for cb in range(KT):
                xht = x_pool.tile([P, N], BF16, tag="xh", name="xht")
                xlt = x_pool.tile([P, N], BF16, tag="xl", name="xlt")
                for t_, ext in ((xht, xh_ext), (xlt, xl_ext)):
                    nc.sync.dma_start(
                        t_[:].rearrange("p (t m) -> p t m", t=KT),
                        ext[:, cb * P : (cb + 1) * P].rearrange(
                            "(t p) m -> p t m", p=P
                        ),
                    )
                pt = ps1.tile([P, FC], F32, tag="ps1", name="pt")
                nmm = 3 * KT
                i = 0
                for rt in range(KT):
                    dh = dh_sb[rt][:, fc * FC : (fc + 1) * FC]
                    dl = dl_sb[rt][:, fc * FC : (fc + 1) * FC]
                    xh = xht[:, rt * P : (rt + 1) * P]
                    xl = xlt[:, rt * P : (rt + 1) * P]
                    for l_, r_ in ((xh, dh), (xh, dl), (xl, dh)):
                        nc.tensor.matmul(
                            pt[:], lhsT=l_, rhs=r_,
                            start=(i == 0), stop=(i == nmm - 1),
                        )
                        i += 1
                # split T on device: th = bf16(T), tl = bf16(T - th)
                th = t_pool.tile([P, FC], BF16, tag="th", name="th")
                tl = t_pool.tile([P, FC], BF16, tag="tl", name="tl")
                tmp = w_pool.tile([P, FC], F32, tag="tmp", name="tmp")
                nc.vector.tensor_copy(th[:], pt[:])
                nc.scalar.copy(tmp[:], th[:])
                nc.vector.tensor_sub(tmp[:], pt[:], tmp[:])
                nc.vector.tensor_copy(tl[:], tmp[:])
                t_tiles.append((th, tl))

            for fb in range(FC // P):
                for g in range(NFC):
                    pz = ps2.tile([P, FC], F32, tag="ps2", name="pz")
                    nmm = 3 * KT
                    i = 0
                    for ct in range(KT):
                        th, tl = t_tiles[ct]
                        dh = dh_sb[ct][:, g * FC : (g + 1) * FC]
                        dl = dl_sb[ct][:, g * FC : (g + 1) * FC]
                        thb = th[:, fb * P : (fb + 1) * P]
                        tlb = tl[:, fb * P : (fb + 1) * P]
                        for l_, r_ in ((thb, dh), (thb, dl), (tlb, dh)):
                            nc.tensor.matmul(
                                pz[:], lhsT=l_, rhs=r_,
                                start=(i == 0), stop=(i == nmm - 1),
                            )
                            i += 1
                    zt = z_pool.tile([P, FC], F32, tag="z", name="zt")
                    nc.vector.tensor_copy(zt[:], pz[:])
                    row0 = (fc * (FC // P) + fb) * P
                    nc.sync.dma_start(
                        z_ext[row0 : row0 + P, g * FC : (g + 1) * FC], zt[:]
                    )

    nc.finalize()
    return nc


_PROGRAM_CACHE: dict = {}


_BUILDERS = {"f32r": _build_f32r, "bfly": _build_bfly, "split": _build_split}


def _get_program(mode: str) -> bass.Bass:
    if mode not in _PROGRAM_CACHE:
        _PROGRAM_CACHE[mode] = _BUILDERS[mode]()
    return _PROGRAM_CACHE[mode]


def _make_in_maps(x: np.ndarray, mode: str):
    if mode == "f32r":
        dr = _round_f32r(_dct_matrix_d())
        return [{"x": _round_f32r(x[i]), "d": dr} for i in range(B)]
    if mode == "bfly":
        n2 = np.arange(H, dtype=np.float64)[:, None]
        j2 = np.arange(H, dtype=np.float64)[None, :]
        de = _round_f32r(np.cos(np.pi * (2 * n2 + 1) * j2 / N).astype(np.float32))
        do = _round_f32r(
            np.cos(np.pi * (2 * n2 + 1) * (2 * j2 + 1) / (2 * N)).astype(
                np.float32
            )
        )
        r = np.eye(P, dtype=np.float32)[::-1].copy()
        maps = []
        for i in range(B):
            xi = np.asarray(x[i], dtype=np.float32)
            xr = xi[::-1]
            maps.append(
                {
                    "u": _round_f32r(xi[:H] + xr[:H]),
                    "v": _round_f32r(xi[:H] - xr[:H]),
                    "de": de,
                    "do": do,
                    "r": r,
                }
            )
        return maps
    dh, dl = _split_bf16(_dct_matrix_d())
    maps = []
    for i in range(B):
        xh, xl = _split_bf16(np.ascontiguousarray(x[i], dtype=np.float32))
        maps.append({"xh": xh, "xl": xl, "dh": dh, "dl": dl})
    return maps


def kernel(x: np.ndarray) -> np.ndarray:
    x = np.asarray(x)
    assert x.shape == (B, N, N), x.shape
    nc = _get_program(MODE)
    in_maps = _make_in_maps(x, MODE)
    res = run_bass_kernel_spmd(nc, in_maps, list(range(B)))
    out = np.stack([res.results[i]["z"] for i in range(B)], axis=0)
    return out.astype(np.float32, copy=False)


# revision 29
# speedup vs baseline: 1.1416x; 1.0037x over previous
"""Trainium2 Bass kernel: batched 2D DCT-II (unnormalized), x: (8, 2048, 2048) f32.

Math: per image X, the unnormalized 2D DCT-II is Z = C @ X @ C^T with
C[k,n] = cos(pi*(2n+1)*k/(2N)).  Let D = C^T.  Using the PE's
out = lhsT.T @ rhs semantics with the *data* as the stationary operand and D as
the moving operand, the two passes chain with no transposes:
    pass 1:  T = X^T @ D      (lhsT = X tiles,  rhs = D)   -> T[c, f]
    pass 2:  Z = T^T @ D      (lhsT = T tiles,  rhs = D)   -> Z = D^T X D = C X C^T

Sharding: batch dim 8 -> one image per NeuronCore (data parallel, no comms).

Dtype modes:
  "f32r"  - single fp32r matmul per term (full PE rate; TF32-like 11-bit
            mantissa operand rounding; ~2e-4 relative-to-absmax error).
  "split" - hi/lo bf16 decomposition, 3 matmuls per term (~5e-6 error, 3x cost).
"""

import numpy as np
from contextlib import ExitStack

import concourse.bass as bass
import concourse.bacc as bacc
import concourse.tile as tile
from concourse import mybir
from concourse.bass_utils import run_bass_kernel_spmd

F32 = mybir.dt.float32
F32R = mybir.dt.float32r
BF16 = mybir.dt.bfloat16

import os

MODE = os.environ.get("DCT_MODE", "bfly")  # "bfly", "f32r", or "split"

B = 8          # batch == n_cores
N = 2048       # image is N x N
P = 128        # partitions
KT = N // P    # 16 k-tiles along any contraction
FC = 512       # chunk width (pass-1 f-chunk, pass-2 g-chunk, PSUM bank)
NFC = N // FC  # 4 chunks
H = N // 2     # butterfly half size
KT2 = H // P   # 8 k-tiles at half contraction


def _round_f32r(a: np.ndarray) -> np.ndarray:
    """fp32r = round-to-nearest, 11 explicit mantissa bits (drop low 12)."""
    b = np.ascontiguousarray(a, dtype=np.float32).view(np.uint32)
    r = ((b + np.uint32(0x800)) & np.uint32(0xFFFFF000)).view(np.float32)
    return r


def _split_bf16(a: np.ndarray):
    import ml_dtypes

    hi = a.astype(ml_dtypes.bfloat16)
    lo = (a - hi.astype(np.float32)).astype(ml_dtypes.bfloat16)
    return hi, lo


def _dct_matrix_d() -> np.ndarray:
    # D[n, k] = cos(pi * (2n+1) * k / (2N)), exact in float64
    n = np.arange(N, dtype=np.float64)[:, None]
    k = np.arange(N, dtype=np.float64)[None, :]
    d = np.cos(np.pi * (2.0 * n + 1.0) * k / (2.0 * N))
    return d.astype(np.float32)


def _build_f32r() -> bass.Bass:
    """fp32r two-pass DCT with the intermediate T round-tripped via DRAM.

    Pass 1 streams X once (one column-block per chain, all 4 f-chunks while
    the block is resident).  T chunks are written back to a DRAM scratch and
    re-streamed as pass-2 stationary tiles.  D stays resident in SBUF.
    """
    nc = bacc.Bacc(None, target_bir_lowering=False)
    x_ext = nc.declare_dram_parameter("x", [N, N], F32R, isOutput=False)
    d_ext = nc.declare_dram_parameter("d", [N, N], F32R, isOutput=False)
    z_ext = nc.declare_dram_parameter("z", [N, N], F32, isOutput=True)

    with ExitStack() as ctx:
        tc = ctx.enter_context(tile.TileContext(nc))
        d_pool = ctx.enter_context(tc.tile_pool(name="d", bufs=1))
        x_pool = ctx.enter_context(tc.tile_pool(name="x", bufs=3))
        t_pool = ctx.enter_context(tc.tile_pool(name="t", bufs=6))
        z_pool = ctx.enter_context(tc.tile_pool(name="z", bufs=3))
        dram = ctx.enter_context(tc.tile_pool(name="dram", bufs=1, space="DRAM"))
        ps1 = ctx.enter_context(tc.tile_pool(name="ps1", bufs=4, space="PSUM"))
        ps2 = ctx.enter_context(tc.tile_pool(name="ps2", bufs=4, space="PSUM"))

        t_dram = dram.tile([N, N], F32R, name="t_dram")

        # First column-block of X loads before D so pass 1 starts early.
        d_sb = [
            d_pool.tile([P, N], F32R, tag=f"d{t}", name=f"d{t}") for t in range(KT)
        ]

        def load_x(cb):
            xt = x_pool.tile([P, N], F32R, tag="x", name="xt")
            nc.sync.dma_start(
                xt[:].rearrange("p (t m) -> p t m", t=KT),
                x_ext[:, cb * P : (cb + 1) * P].rearrange("(t p) m -> p t m", p=P),
            )
            return xt

        x0 = load_x(0)
        # D f-chunk 0 for all 16 row-tiles (pass-1 chain 0 needs only these)
        for fcol in range(NFC):
            for t in range(KT):
                nc.sync.dma_start(
                    d_sb[t][:, fcol * FC : (fcol + 1) * FC],
                    d_ext[t * P : (t + 1) * P, fcol * FC : (fcol + 1) * FC],
                )
            if fcol == 0:
                # remaining D chunks stream behind pass-1 compute
                pass

        # ---- pass 1: per column-block cb, all f-chunks: T[cb,:] = (X^T D)[cb,:]
        for cb in range(KT):
            xt = x0 if cb == 0 else load_x(cb)
            for fc in range(NFC):
                pt = ps1.tile([P, FC], F32, tag="ps1", name="pt")
                for rt in range(KT):
                    nc.tensor.matmul(
                        pt[:],
                        lhsT=xt[:, rt * P : (rt + 1) * P],
                        rhs=d_sb[rt][:, fc * FC : (fc + 1) * FC],
                        start=(rt == 0),
                        stop=(rt == KT - 1),
                    )
                tt = t_pool.tile([P, FC], F32R, tag="t", name="tt")
                nc.vector.tensor_copy(tt[:], pt[:])
                nc.scalar.dma_start(
                    t_dram[cb * P : (cb + 1) * P, fc * FC : (fc + 1) * FC], tt[:]
                )

        # ---- pass 2: per f-block fb: Z[fb,:] = (T^T D)[fb,:]
        for fb in range(KT):
            tf = x_pool.tile([P, N], F32R, tag="x", name="tf")
            nc.sync.dma_start(
                tf[:].rearrange("p (t m) -> p t m", t=KT),
                t_dram[:, fb * P : (fb + 1) * P].rearrange("(t p) m -> p t m", p=P),
            )
            for g in range(NFC):
                pz = ps2.tile([P, FC], F32, tag="ps2", name="pz")
                for ct in range(KT):
                    nc.tensor.matmul(
                        pz[:],
                        lhsT=tf[:, ct * P : (ct + 1) * P],
                        rhs=d_sb[ct][:, g * FC : (g + 1) * FC],
                        start=(ct == 0),
                        stop=(ct == KT - 1),
                    )
                zt = z_pool.tile([P, FC], F32, tag="z", name="zt")
                nc.vector.tensor_copy(zt[:], pz[:])
                nc.scalar.dma_start(
                    z_ext[fb * P : (fb + 1) * P, g * FC : (g + 1) * FC], zt[:]
                )

    nc.finalize()
    return nc


def _build_bfly() -> bass.Bass:
    """Radix-2 even/odd DCT factorization in fp32r: each 1D DCT-II of size N
    becomes two size-N/2 cosine transforms of the folded sequences
    u = x_top + reverse(x_bot), v = x_top - reverse(x_bot):
        y[2j]   = sum_{n<H} u[n] De[n, j],   De[n,j] = cos(pi (2n+1) j / N)
        y[2j+1] = sum_{n<H} v[n] Do[n, j],   Do[n,j] = cos(pi (2n+1)(2j+1) / 2N)
    Halves the matmul work per pass.  Pass-1 folding is done on the host
    (u/v uploaded); pass-2 folding of the intermediate T is done by DVE with a
    reversed-row DMA load.  Outputs are de-interleaved on chip (strided DVE
    writes) + stride-2-row DMA stores, so all DRAM traffic stays contiguous
    per partition.
    """
    nc = bacc.Bacc(None, target_bir_lowering=False)
    u_ext = nc.declare_dram_parameter("u", [H, N], F32R, isOutput=False)
    v_ext = nc.declare_dram_parameter("v", [H, N], F32R, isOutput=False)
    de_ext = nc.declare_dram_parameter("de", [H, H], F32R, isOutput=False)
    do_ext = nc.declare_dram_parameter("do", [H, H], F32R, isOutput=False)
    r_ext = nc.declare_dram_parameter("r", [P, P], F32R, isOutput=False)
    z_ext = nc.declare_dram_parameter("z", [N, N], F32, isOutput=True)

    with ExitStack() as ctx:
        tc = ctx.enter_context(tile.TileContext(nc))
        d_pool = ctx.enter_context(tc.tile_pool(name="d", bufs=1))
        x_pool = ctx.enter_context(tc.tile_pool(name="x", bufs=4))
        t_pool = ctx.enter_context(tc.tile_pool(name="t", bufs=4))
        b_pool = ctx.enter_context(tc.tile_pool(name="b", bufs=4))
        z_pool = ctx.enter_context(tc.tile_pool(name="z", bufs=3))
        dram = ctx.enter_context(tc.tile_pool(name="dram", bufs=1, space="DRAM"))
        # PSUM: "acc" slots [P, H] f32 (2 banks) shared between pass-1
        # accumulators and pass-2 reversal matmuls (2x2 banks); pass-2 output
        # chains share one 4-slot pool (4 banks) so a slow de-interleave copy
        # on either engine doesn't stall the next chain. 8 banks total.
        ps1 = ctx.enter_context(tc.tile_pool(name="ps1", bufs=2, space="PSUM"))
        ps2 = ctx.enter_context(tc.tile_pool(name="ps2", bufs=4, space="PSUM"))

        # T in blocked layout: cols [0,H) = even outputs, [H,2H) = odd
        t_dram = dram.tile([N, N], F32R, name="t_dram")

        de_sb = [
            d_pool.tile([P, H], F32R, tag=f"de{t}", name=f"de{t}")
            for t in range(KT2)
        ]
        do_sb = [
            d_pool.tile([P, H], F32R, tag=f"do{t}", name=f"do{t}")
            for t in range(KT2)
        ]

        def load_block(ext, cb, tag):
            w = x_pool.tile([P, H], F32R, tag=tag, name="w_" + tag)
            nc.sync.dma_start(
                w[:].rearrange("p (t m) -> p t m", t=KT2),
                ext[:, cb * P : (cb + 1) * P].rearrange("(t p) m -> p t m", p=P),
            )
            return w

        # loads in exact first-consumption order: u0, de jc0, de jc1, v0,
        # do jc0, do jc1; the pass-2 reversal matrix r last.
        u0 = load_block(u_ext, 0, "u")
        for jc in range(2):
            for t in range(KT2):
                nc.sync.dma_start(
                    de_sb[t][:, jc * FC : (jc + 1) * FC],
                    de_ext[t * P : (t + 1) * P, jc * FC : (jc + 1) * FC],
                )
        v0 = load_block(v_ext, 0, "v")
        for jc in range(2):
            for t in range(KT2):
                nc.sync.dma_start(
                    do_sb[t][:, jc * FC : (jc + 1) * FC],
                    do_ext[t * P : (t + 1) * P, jc * FC : (jc + 1) * FC],
                )
        r_sb = d_pool.tile([P, P], F32R, tag="r", name="r_sb")
        nc.sync.dma_start(r_sb[:], r_ext[:])

        # ---- pass 1: T_blk[cb, :] ----
        for cb in range(KT):
            ut = u0 if cb == 0 else load_block(u_ext, cb, "u")
            vt = v0 if cb == 0 else load_block(v_ext, cb, "v")
            for half, (wt, dsb) in enumerate(((ut, de_sb), (vt, do_sb))):
                for jc in range(2):
                    pt = ps1.tile([P, FC], F32, tag="acc", name="pt")
                    for rt in range(KT2):
                        nc.tensor.matmul(
                            pt[:],
                            lhsT=wt[:, rt * P : (rt + 1) * P],
                            rhs=dsb[rt][:, jc * FC : (jc + 1) * FC],
                            start=(rt == 0),
                            stop=(rt == KT2 - 1),
                        )
                    tt = t_pool.tile([P, FC], F32R, tag="t", name="tt")
                    nc.vector.tensor_copy(tt[:], pt[:])
                    col0 = half * H + jc * FC
                    nc.scalar.dma_start(
                        t_dram[cb * P : (cb + 1) * P, col0 : col0 + FC], tt[:]
                    )

        # ---- pass 2: fold T over rows, transform, de-interleave out ----
        # bot_rev[c', f] = T[2047-c', f]: partition reversal via one PE matmul
        # with the reversal permutation R (out[m,n] = bot[127-m, n]); the
        # tile-order flip (ct -> 7-ct) via a reversed free-dim view in the add.
        # Software-pipelined: loads run 3 blocks ahead, reversal matmul + DVE
        # fold 2 ahead, so block fb's chains never wait on its fold.
        folded: dict = {}

        def p2_load(fb):
            top = b_pool.tile([P, H], F32R, tag="top", name="top")
            nc.sync.dma_start(
                top[:].rearrange("p (t m) -> p t m", t=KT2),
                t_dram[0:H, fb * P : (fb + 1) * P].rearrange(
                    "(t p) m -> p t m", p=P
                ),
            )
            bot = b_pool.tile([P, H], F32R, tag="bot", name="bot")
            nc.sync.dma_start(
                bot[:].rearrange("p (t m) -> p t m", t=KT2),
                t_dram[H:N, fb * P : (fb + 1) * P].rearrange(
                    "(t p) m -> p t m", p=P
                ),
            )
            folded[fb] = (top, bot)

        def p2_fold(fb):
            top, bot = folded[fb]
            pr = ps1.tile([P, H], F32, tag="acc", name="pr")
            for hc in range(2):
                nc.tensor.matmul(
                    pr[:, hc * FC : (hc + 1) * FC],
                    lhsT=r_sb[:],
                    rhs=bot[:, hc * FC : (hc + 1) * FC],
                    start=True,
                    stop=True,
                )
            # view with tile index reversed: pr_rev[p, ct, m] = pr[p, 7-ct, m]
            pr_rev = pr[:].rearrange("p (t m) -> p t m", t=KT2)[:, ::-1, :]
            top3 = top[:].rearrange("p (t m) -> p t m", t=KT2)
            u2 = b_pool.tile([P, H], F32R, tag="u2", name="u2")
            nc.vector.tensor_add(
                u2[:].rearrange("p (t m) -> p t m", t=KT2), top3, pr_rev
            )
            v2 = b_pool.tile([P, H], F32R, tag="v2", name="v2")
            nc.vector.tensor_sub(
                v2[:].rearrange("p (t m) -> p t m", t=KT2), top3, pr_rev
            )
            folded[fb] = (u2, v2)

        p2_load(0)
        p2_load(1)
        p2_fold(0)
        p2_load(2)
        p2_fold(1)
        for fb in range(KT):
            u2, v2 = folded.pop(fb)
            # f_blk block fb -> actual Z rows (de-interleave rows via stride 2)
            if fb < KT2:
                row0 = 2 * fb * P
                row_stop = row0 + 2 * P
            else:
                row0 = 2 * (fb - KT2) * P + 1
                row_stop = row0 + 2 * P - 1
            for jc in range(2):
                pe_ = ps2.tile([P, FC], F32, tag="zacc", name="pe_")
                for ct in range(KT2):
                    nc.tensor.matmul(
                        pe_[:],
                        lhsT=u2[:, ct * P : (ct + 1) * P],
                        rhs=de_sb[ct][:, jc * FC : (jc + 1) * FC],
                        start=(ct == 0),
                        stop=(ct == KT2 - 1),
                    )
                po_ = ps2.tile([P, FC], F32, tag="zacc", name="po_")
                for ct in range(KT2):
                    nc.tensor.matmul(
                        po_[:],
                        lhsT=v2[:, ct * P : (ct + 1) * P],
                        rhs=do_sb[ct][:, jc * FC : (jc + 1) * FC],
                        start=(ct == 0),
                        stop=(ct == KT2 - 1),
                    )
                zt = z_pool.tile([P, 2 * FC], F32, tag="z", name="zt")
                nc.scalar.copy(zt[:, 0 : 2 * FC : 2], pe_[:])
                nc.vector.tensor_copy(zt[:, 1 : 2 * FC : 2], po_[:])
                nc.scalar.dma_start(
                    z_ext[row0:row_stop:2, jc * 2 * FC : (jc + 1) * 2 * FC],
                    zt[:],
                )
            if fb + 3 < KT:
                p2_load(fb + 3)
            if fb + 2 < KT:
                p2_fold(fb + 2)

    nc.finalize()
    return nc


def _build_split() -> bass.Bass:
    """hi/lo bf16 decomposition: each logical matmul = 3 bf16 matmuls
    (Xh Dh + Xh Dl + Xl Dh), accumulated in the same PSUM chain."""
    nc = bacc.Bacc(None, target_bir_lowering=False)
    xh_ext = nc.declare_dram_parameter("xh", [N, N], BF16, isOutput=False)
    xl_ext = nc.declare_dram_parameter("xl", [N, N], BF16, isOutput=False)
    dh_ext = nc.declare_dram_parameter("dh", [N, N], BF16, isOutput=False)
    dl_ext = nc.declare_dram_parameter("dl", [N, N], BF16, isOutput=False)
    z_ext = nc.declare_dram_parameter("z", [N, N], F32, isOutput=True)

    with ExitStack() as ctx:
        tc = ctx.enter_context(tile.TileContext(nc))
        d_pool = ctx.enter_context(tc.tile_pool(name="d", bufs=1))
        x_pool = ctx.enter_context(tc.tile_pool(name="x", bufs=3))
        w_pool = ctx.enter_context(tc.tile_pool(name="w", bufs=3))
        t_pool = ctx.enter_context(tc.tile_pool(name="t", bufs=KT))
        z_pool = ctx.enter_context(tc.tile_pool(name="z", bufs=3))
        ps1 = ctx.enter_context(tc.tile_pool(name="ps1", bufs=4, space="PSUM"))
        ps2 = ctx.enter_context(tc.tile_pool(name="ps2", bufs=4, space="PSUM"))

        dh_sb = [
            d_pool.tile([P, N], BF16, tag=f"dh{t}", name=f"dh{t}")
            for t in range(KT)
        ]
        dl_sb = [
            d_pool.tile([P, N], BF16, tag=f"dl{t}", name=f"dl{t}")
            for t in range(KT)
        ]
        for fcol in range(NFC):
            for t in range(KT):
                nc.sync.dma_start(
                    dh_sb[t][:, fcol * FC : (fcol + 1) * FC],
                    dh_ext[t * P : (t + 1) * P, fcol * FC : (fcol + 1) * FC],
                )
                nc.sync.dma_start(
                    dl_sb[t][:, fcol * FC : (fcol + 1) * FC],
                    dl_ext[t * P : (t + 1) * P, fcol * FC : (fcol + 1) * FC],
                )

        for fc in range(NFC):
            t_tiles = []
            for cb in range(KT):
                xht = x_pool.tile([P, N], BF16, tag="xh", name="xht")
                xlt = x_pool.tile([P, N], BF16, tag="xl", name="xlt")
                for t_, ext in ((xht, xh_ext), (xlt, xl_ext)):
                    nc.sync.dma_start(
                        t_[:].rearrange("p (t m) -> p t m", t=KT),
                        ext[:, cb * P : (cb + 1) * P].rearrange(
                            "(t p) m -> p t m", p=P
                        ),
                    )
                pt = ps1.tile([P, FC], F32, tag="ps1", name="pt")
                nmm = 3 * KT
                i = 0
                for rt in range(KT):
                    dh = dh_sb[rt][:, fc * FC : (fc + 1) * FC]
                    dl = dl_sb[rt][:, fc * FC : (fc + 1) * FC]
                    xh = xht[:, rt * P : (rt + 1) * P]
                    xl = xlt[:, rt * P : (rt + 1) * P]
                    for l_, r_ in ((xh, dh), (xh, dl), (xl, dh)):
                        nc.tensor.matmul(
                            pt[:], lhsT=l_, rhs=r_,
                            start=(i == 0), stop=(i == nmm - 1),
                        )
                        i += 1
                # split T on device: th = bf16(T), tl = bf16(T - th)
                th = t_pool.tile([P, FC], BF16, tag="th", name="th")
                tl = t_pool.tile([P, FC], BF16, tag="tl", name="tl")
                tmp = w_pool.tile([P, FC], F32, tag="tmp", name="tmp")
                nc.vector.tensor_copy(th[:], pt[:])
                nc.scalar.copy(tmp[:], th[:])
                nc.vector.tensor_sub(tmp[:], pt[:], tmp[:])
                nc.vector.tensor_copy(tl[:], tmp[:])
                t_tiles.append((th, tl))

            for fb in range(FC // P):
                for g in range(NFC):
                    pz = ps2.tile([P, FC], F32, tag="ps2", name="pz")
                    nmm = 3 * KT
                    i = 0
                    for ct in range(KT):
                        th, tl = t_tiles[ct]
                        dh = dh_sb[ct][:, g * FC : (g + 1) * FC]
                        dl = dl_sb[ct][:, g * FC : (g + 1) * FC]
                        thb = th[:, fb * P : (fb + 1) * P]
                        tlb = tl[:, fb * P : (fb + 1) * P]
                        for l_, r_ in ((thb, dh), (thb, dl), (tlb, dh)):
                            nc.tensor.matmul(
                                pz[:], lhsT=l_, rhs=r_,
                                start=(i == 0), stop=(i == nmm - 1),
                            )
                            i += 1
                    zt = z_pool.tile([P, FC], F32, tag="z", name="zt")
                    nc.vector.tensor_copy(zt[:], pz[:])
                    row0 = (fc * (FC // P) + fb) * P
                    nc.sync.dma_start(
                        z_ext[row0 : row0 + P, g * FC : (g + 1) * FC], zt[:]
                    )

    nc.finalize()
    return nc


_PROGRAM_CACHE: dict = {}


_BUILDERS = {"f32r": _build_f32r, "bfly": _build_bfly, "split": _build_split}


def _get_program(mode: str) -> bass.Bass:
    if mode not in _PROGRAM_CACHE:
        _PROGRAM_CACHE[mode] = _BUILDERS[mode]()
    return _PROGRAM_CACHE[mode]


def _make_in_maps(x: np.ndarray, mode: str):
    if mode == "f32r":
        dr = _round_f32r(_dct_matrix_d())
        return [{"x": _round_f32r(x[i]), "d": dr} for i in range(B)]
    if mode == "bfly":
        n2 = np.arange(H, dtype=np.float64)[:, None]
        j2 = np.arange(H, dtype=np.float64)[None, :]
        de = _round_f32r(np.cos(np.pi * (2 * n2 + 1) * j2 / N).astype(np.float32))
        do = _round_f32r(
            np.cos(np.pi * (2 * n2 + 1) * (2 * j2 + 1) / (2 * N)).astype(
                np.float32
            )
        )
        r = np.eye(P, dtype=np.float32)[::-1].copy()
        maps = []
        for i in range(B):
            xi = np.asarray(x[i], dtype=np.float32)
            xr = xi[::-1]
            maps.append(
                {
                    "u": _round_f32r(xi[:H] + xr[:H]),
                    "v": _round_f32r(xi[:H] - xr[:H]),
                    "de": de,
                    "do": do,
                    "r": r,
                }
            )
        return maps
    dh, dl = _split_bf16(_dct_matrix_d())
    maps = []
    for i in range(B):
        xh, xl = _split_bf16(np.ascontiguousarray(x[i], dtype=np.float32))
        maps.append({"xh": xh, "xl": xl, "dh": dh, "dl": dl})
    return maps


def kernel(x: np.ndarray) -> np.ndarray:
    x = np.asarray(x)
    assert x.shape == (B, N, N), x.shape
    nc = _get_program(MODE)
    in_maps = _make_in_maps(x, MODE)
    res = run_bass_kernel_spmd(nc, in_maps, list(range(B)))
    out = np.stack([res.results[i]["z"] for i in range(B)], axis=0)
    return out.astype(np.float32, copy=False)
